# revision 29
# baseline (speedup 1.0000x reference)
"""Trainium2 Bass kernel for the multi-task ActorNetwork (moe_routing).

Architecture (reference): per-sample expert routing over G=8 tasks:
    h1 = relu(x @ W1[idx] + b1[idx])     x:[B,376]  W1:[8,376,400]
    hf = relu(h1 @ W2 + b2)              W2:[400,300]
    a  = tanh(hf @ W3[idx] + b3[idx])    W3:[8,300,17]

Strategy: idx is sorted, and G == n_cores == 8, so we route on the HOST:
core g receives exactly the contiguous rows with idx == g (zero-padded to a
common BM), plus only ITS expert weights. Each core then runs a dense 3-layer
MLP -- no device-side routing, no collectives, and none of the 8x dense
compute the reference does.

Numerics: fp16 operands with fp32 PSUM accumulation; measured end-to-end
max-abs error vs the fp32 reference ~5e-3 on unit-scale outputs.

Matmul cost on the PE is (output free size) x (cycles/row), so the layout is
chosen to minimize streamed output elements:
  * L1/L2 main tiles (128-feature groups) run feature-major: the contraction
    dim sits on SBUF partitions and the 512-sample chunk streams as the
    moving dim.
  * The ragged remainders (h1's last 16 features, hf's last 44) run
    BATCH-major: out[b_slice, F] = x_sliceT.T @ W[:, rem] streams only F
    elements per pass; one PE transpose per 128-sample slice (through fp16
    PSUM) restores the feature-major layout the next layer needs.  The
    current chunk's 16-col block and the PREVIOUS chunk's 44-col block are
    relu'd into one [128, 4, 109] SBUF tile and share the same transpose
    (the hf44 block sits at column 64 because 45-partition engine reads must
    start at partition 0 or 64).
  * L3 is fully batch-major (17 outputs): lhsT = a 128-column slice of hfT,
    rhs = W3 -- ~17 cycles per slice-pass instead of 512 per K-pass.  All L3
    matmuls of a chunk form ONE PSUM accumulation group in one bank writing
    disjoint 17-column slices.
  * ALL biases ride spare contraction rows: x carries a ones-row at 376 and
    W1's K-slab carries b1 there; the transposed remainders carry a ones-row
    (memset column of the [128,4,62] tile) matched by b2/b3 rows in the
    augmented W2/W3 K-slabs.  No bias operands, no rank-1 bias matmuls.

The last two chunks run the plain feature-major path (K-outer over three L2
groups, borrowing the then-idle psbm bank): slightly more PE work, but the
drain has no transpose/copy chain to serialize on, which keeps the tail to
tanh -> out-DMA -> barrier (~2.8us, dominated by the modeled DMA latency).

Engine split: PE matmuls+transposes; ACT: L1 relu x2, BM16 relu, hf44T copy,
L3 tanh; DVE: L1 relu x1, L2 relu x2, BM44 relu, h1remT copy; Pool: weight
DMAs; SP: x-chunk streaming + out DMAs.  A dummy activation at t~0.5us
preloads the ACT function table off the critical path.
"""

import sys

if "/opt/trn_rl_repo" not in sys.path:
    sys.path.insert(0, "/opt/trn_rl_repo")

from contextlib import ExitStack

import numpy as np

import concourse.bass as bass
import concourse.mybir as mybir
from concourse.bass_utils import run_bass_kernel_spmd
from concourse.tile import TileContext

D, G, H1, H2, A = 376, 8, 400, 300, 17
P = 128
NCORES = 8
F16 = mybir.dt.float16
F32 = mybir.dt.float32

F1R = H1 - 3 * P  # 16: L1 feature remainder
F2R = H2 - 2 * P  # 44: L2 feature remainder
HF44_OFF = 64  # hf44 block column offset (45-partition reads must start at 0/64)
BMC = HF44_OFF + F2R + 1  # 109: combined-transpose tile columns

NK1, NK2, NK3 = 3, 4, 3
KS1 = [128, 128, D - 256]  # x contraction slabs (120-slab holds the ones row)
KS2 = [128, 128, 128, F1R + 1]  # L2 slabs; last = h1rem + ones(b2) row
KS3 = [128, 128, F2R + 1]  # L3 slabs; last = hf44 + ones(b3) row
M1 = [(0, 128), (128, 128), (256, 128), (384, F1R)]
M2 = [(0, 128), (128, 128), (256, F2R)]

_nc_cache = {}
last_run = None  # BassKernelResults of the most recent launch (for profiling)
_last_in_maps = None  # per-core input dicts of the most recent launch

# Cross-core overflow rebalancing: when some experts exceed A_CAP=4096 rows,
# every core runs 8 full own-expert chunks plus one small B-column chunk that
# applies a DONOR expert's weight set; the host routes hot experts' overflow
# rows into other cores' B-chunks.  _plan maps the nominal BM (what the test
# harness computes from counts) to the B width so _build stays reproducible
# from (BM,) alone.
A_CAP = 8 * 512
_plan = {}

_nop_counter = [0]


def _chunks(total, step):
    return [(o, min(step, total - o)) for o in range(0, total, step)]


def _legalize_wait_counts(nc):
    """This container's walrus encodes at most ONE sync-wait per instruction
    (DMA pseudo-instructions especially). Tile freely emits several. Sequencers
    are in-order, so hoisting the surplus waits onto same-engine NoOps placed
    immediately before the instruction is semantics-preserving."""
    for fn in nc.m.functions:
        for bb in fn.blocks:
            insts = list(bb.instructions)
            out = []
            changed = False
            for inst in insts:
                si = inst.sync_info
                waits = list(si.on_wait) if si is not None and si.on_wait else []
                if len(waits) > 1:
                    changed = True
                    for w in waits[:-1]:
                        _nop_counter[0] += 1
                        nop = mybir.InstNoOp(
                            name=f"waitsplit_nop_{_nop_counter[0]}",
                            engine=inst.engine,
                            ins=[],
                            outs=[],
                            sync_info=mybir.SyncInfo(on_wait=[w], on_update=[]),
                        )
                        out.append(nop)
                    si.on_wait = waits[-1:]
                out.append(inst)
            if changed:
                bb.instructions = out
    return nc


def _build(BM, legalize=True, reps=1):
    """Bass program for one core: dense 3-layer MLP over BM rows.

    reps>1 wraps the body in a hardware For_i loop (benchmarking only)."""
    B = _plan.get(BM, 0)
    if B:
        XW = A_CAP + B  # columns of xP; the last chunk is the B donor chunk
        bchunks = _chunks(A_CAP, 512) + [(A_CAP, B)]
        NS = A_CAP // P + 1
    else:
        XW = BM
        bchunks = _chunks(BM, 512)
        NS = BM // P
    nc = bass.Bass()
    xP = nc.declare_dram_parameter("xP", [P, NK1, XW], F16, isOutput=False)
    w1 = nc.declare_dram_parameter("w1", [P, NK1, H1], F16, isOutput=False)
    w2 = nc.declare_dram_parameter("w2", [P, NK2, H2], F16, isOutput=False)
    w3 = nc.declare_dram_parameter("w3", [P, NK3, A], F16, isOutput=False)
    if B:
        w1b = nc.declare_dram_parameter("w1b", [P, NK1, H1], F16, isOutput=False)
        w3b = nc.declare_dram_parameter("w3b", [P, NK3, A], F16, isOutput=False)
    ident = nc.declare_dram_parameter("ident", [P, P], F16, isOutput=False)
    # out[p, s, a] = action a of sample s*128 + p (host re-interleaves)
    out = nc.declare_dram_parameter("out", [P, NS, A], F32, isOutput=True)

    Relu = mybir.ActivationFunctionType.Relu
    Tanh = mybir.ActivationFunctionType.Tanh
    Add = mybir.AluOpType.add
    Max = mybir.AluOpType.max

    with TileContext(nc) as tc, ExitStack() as ctx:
        wpool = ctx.enter_context(tc.tile_pool(name="w", bufs=1))
        xpool = ctx.enter_context(tc.tile_pool(name="x", bufs=3))
        h1pool = ctx.enter_context(tc.tile_pool(name="h1", bufs=3))
        hfpool = ctx.enter_context(tc.tile_pool(name="hf", bufs=3))
        bmpool = ctx.enter_context(tc.tile_pool(name="bm", bufs=2))
        opool = ctx.enter_context(tc.tile_pool(name="o", bufs=3))
        ps1 = ctx.enter_context(tc.tile_pool(name="ps1", bufs=3, space="PSUM"))
        ps2 = ctx.enter_context(tc.tile_pool(name="ps2", bufs=2, space="PSUM"))
        psbm = ctx.enter_context(tc.tile_pool(name="psbm", bufs=1, space="PSUM"))
        pst = ctx.enter_context(tc.tile_pool(name="pst", bufs=1, space="PSUM"))
        ps3 = ctx.enter_context(tc.tile_pool(name="ps3", bufs=1, space="PSUM"))

        def load_weights(param, nk, ncols, name, eng):
            tiles = []
            for ki in range(nk):
                t = wpool.tile([P, ncols], F16, tag=f"{name}_{ki}")
                eng.dma_start(out=t[:, :], in_=param[:, ki, :])
                tiles.append(t)
            return tiles

        # Ring latency tuning: w1's K0 slab rides the ACT HWDGE ring (sem at
        # ~2.4us, the earliest possible) so the very first matmul is not
        # gated on the slower Pool ring; everything else streams on Pool in
        # deadline order, leaving ACT free for chunk-0's evictions.
        w1_t = [None] * NK1
        t0 = wpool.tile([P, H1], F16, tag="w1_0")
        nc.scalar.dma_start(out=t0[:, :], in_=w1[:, 0, :])
        w1_t[0] = t0
        for ki in (1, 2):
            t = wpool.tile([P, H1], F16, tag=f"w1_{ki}", name=f"w1t_{ki}")
            nc.gpsimd.dma_start(out=t[:, :], in_=w1[:, ki, :])
            w1_t[ki] = t
        ident_t = wpool.tile([P, P], F16, tag="ident")
        nc.gpsimd.dma_start(out=ident_t[:, :], in_=ident[:, :])
        w2_t = load_weights(w2, NK2, H2, "w2", nc.gpsimd)
        w3_t = load_weights(w3, NK3, A, "w3", nc.gpsimd)

        # preload the ACT function table off the critical path (the first
        # activation otherwise pays ~1.4us mid-stream)
        seed_t = wpool.tile([1, 1], F32, tag="seed")
        nc.vector.memset(seed_t[:, :], 0.0)
        actw_t = wpool.tile([1, 1], F32, tag="actw")
        nc.scalar.activation(actw_t[:, :], seed_t[:, :], Relu)

        if B:
            w1b_t = load_weights(w1b, NK1, H1, "w1b", nc.gpsimd)
            w3b_t = load_weights(w3b, NK3, A, "w3b", nc.gpsimd)
        else:
            w1b_t, w3b_t = w1_t, w3_t

        def emit_chunk(ci, b0, nb, prev):
            # the last TWO chunks run plain feature-major: slightly more PE
            # work, but the drain has no transpose/copy chain to serialize on
            bm = nb == 512 and ci < len(bchunks) - 1
            ns = (nb + P - 1) // P
            donor = B and ci == len(bchunks) - 1
            w1s = w1b_t if donor else w1_t
            w3s = w3b_t if donor else w3_t

            # ---- x DMA (chunk 0: per-K-slab so the first passes start early)
            xt = xpool.tile([P, NK1, 512], F16, tag="x")
            if ci == 0:
                for ki in range(NK1):
                    nc.sync.dma_start(out=xt[:, ki, :nb], in_=xP[:, ki, b0 : b0 + nb])
            else:
                nc.sync.dma_start(out=xt[:, :, :nb], in_=xP[:, :, b0 : b0 + nb])

            # ---- P1: L1 feature-major tiles (b1 rides x's ones row) ----
            nfm1 = 3 if bm else len(M1)
            pts1 = [
                ps1.tile([P, 512], F32, tag="ps1", name=f"ps1_{ci}_{i}")
                for i in range(min(nfm1, 3))
            ]
            if not bm and nfm1 == 4:
                # last/partial chunk: M-outer, M3 reuses M0's bank after its
                # eviction (sequential groups -- no WAR deadlock)
                pts1.append(ps1.tile([P, 512], F32, tag="ps1", name=f"ps1_{ci}_3"))
            h1_t = [None] * NK2
            order1 = [(ki, mi) for mi in range(nfm1) for ki in range(NK1)]
            for ki, mi in order1:
                m0, ms = M1[mi]
                nc.tensor.matmul(
                    pts1[mi][:ms, :nb],
                    w1s[ki][:, m0 : m0 + ms],
                    xt[:, ki, :nb],
                    start=(ki == 0),
                    stop=(ki == NK1 - 1),
                )
                if ki == NK1 - 1:
                    if mi == 3:
                        # FM remainder (partial chunk): augmented ones row
                        # carries b2 into the next layer's contraction
                        ht = h1pool.tile([F1R + 1, nb], F16, tag="h1_3")
                        # engine ops must start at partition 0: fill the whole
                        # tile with ones, the eviction overwrites rows 0:16
                        nc.vector.memset(ht[: F1R + 1, :nb], 1.0)
                        nc.vector.tensor_scalar(
                            ht[:ms, :nb], pts1[mi][:ms, :nb], 0.0, None, op0=Max
                        )
                    else:
                        ht = h1pool.tile([ms, nb], F16, tag=f"h1_{mi}")
                        on_act = mi < 2 if bm else mi % 2 == 0
                        if on_act:
                            nc.scalar.activation(ht[:ms, :nb], pts1[mi][:ms, :nb], Relu)
                        else:
                            nc.vector.tensor_scalar(
                                ht[:ms, :nb], pts1[mi][:ms, :nb], 0.0, None, op0=Max
                            )
                    h1_t[mi] = ht

            # ---- P2: L1 batch-major remainder (16 features) + the combined
            #      transpose staging tile ----
            ptbm = None
            bmc_sb = bmpool.tile([P, 4, BMC], F16, tag="bmc")
            # ones columns (transpose into the b2/b3 contraction rows of
            # h1remT/hf44T); cheap [128,4] writes, re-set each rotation so
            # CoreSim's fresh-tile NaN canaries never leak into the transpose
            nc.vector.memset(bmc_sb[:, :, F1R : F1R + 1], 1.0)
            nc.vector.memset(bmc_sb[:, :, BMC - 1 : BMC], 1.0)
            if bm:
                # psbm bank: cols [0,4*F1R) = BM16, [4*F1R,..) = BM44; ONE
                # accumulation group from the first BM16 mm to the last BM44
                # mm (each slice's first write lands on pending-zero bytes).
                ptbm = psbm.tile([P, 512], F32, tag="psbm", name=f"ptbm_{ci}")
                for s in range(4):
                    o = ptbm[:, s * F1R : (s + 1) * F1R]
                    c0 = s * P
                    for ki in range(NK1):
                        nc.tensor.matmul(
                            o,
                            xt[:, ki, c0 : c0 + P],
                            w1s[ki][:, 3 * P : H1],
                            start=(s == 0 and ki == 0),
                            stop=False,
                            skip_group_check=True,
                        )
                bm1v = bmc_sb[:, :, :F1R]
                nc.scalar.activation(bm1v, ptbm[:, : 4 * F1R], Relu)

            # ---- P2.5: previous chunk's BM44 relu into the combined tile --
            if prev is not None and prev["ptbm"] is not None:
                nc.vector.tensor_scalar(
                    bmc_sb[:, :, HF44_OFF : HF44_OFF + F2R],
                    prev["ptbm"][:, 4 * F1R : 4 * (F1R + F2R)],
                    0.0,
                    None,
                    op0=Max,
                )

            # ---- P4..P7: L2 feature-major K-rounds 0..2 (+T after K1) ----
            nfm2 = 2 if bm else len(M2)
            m2fm = M2[:nfm2]
            pts2 = [
                ps2.tile([P, 512], F32, tag="ps2", name=f"ps2_{ci}_{i}")
                for i in range(min(nfm2, 2))
            ]
            if nfm2 == 3:
                # FM chunks borrow the (idle at that point) psbm bank for the
                # third concurrent K-outer group
                pts2.append(psbm.tile([P, 512], F32, tag="psbm", name=f"psd_{ci}"))

            def l2_round(ki):
                ks = KS2[ki]
                for mi, (m0, ms) in enumerate(m2fm):
                    nc.tensor.matmul(
                        pts2[mi][:ms, :nb],
                        w2_t[ki][:ks, m0 : m0 + ms],
                        h1_t[ki][:ks, :nb],
                        start=(ki == 0),
                        stop=(ki == NK2 - 1),
                    )

            def emit_transposes():
                # one [128,109] transpose per slice: rows 0:17 become the
                # augmented h1remT (this chunk), rows 64:109 the augmented
                # hf44T (previous chunk; 45-partition engine reads must start
                # at partition 0 or 64, hence the column gap)
                ptt = pst.tile([BMC, 512], F16, tag="pst")
                for s in range(4):
                    nc.tensor.transpose(
                        ptt[:BMC, s * P : (s + 1) * P],
                        bmc_sb[:, s, :],
                        ident_t[:, :],
                    )
                if prev is not None and prev["ptbm"] is not None:
                    # ACT as copy engine: values are post-relu/ones (>=0)
                    nc.scalar.activation(
                        prev["hf_t"][2][: F2R + 1, :],
                        ptt[HF44_OFF : HF44_OFF + F2R + 1, :],
                        Relu,
                    )
                if bm:
                    h1r = h1pool.tile([F1R + 1, 512], F16, tag="h1r")
                    nc.vector.tensor_scalar(
                        h1r[:, :], ptt[: F1R + 1, :], 0.0, None, op0=Add
                    )
                    h1_t[3] = h1r

            if bm:
                l2_round(0)
                l2_round(1)
                emit_transposes()
                l2_round(2)
            else:
                emit_transposes()
                # drain chunks: the FM-512 chunk runs M-outer so each hf tile
                # stops (and evicts) a third of the chunk early, spreading the
                # big [128,512] evictions instead of bunching them at the end;
                # the tiny B chunk keeps K-outer
                hf_t = [None] * len(M2)
                if nb == 512:
                    order2 = [(ki, mi) for mi in range(len(m2fm)) for ki in range(NK2)]
                else:
                    order2 = [(ki, mi) for ki in range(NK2) for mi in range(len(m2fm))]
                for ki, mi in order2:
                    ks = KS2[ki]
                    if True:
                        m0, ms = m2fm[mi]
                        nc.tensor.matmul(
                            pts2[mi][:ms, :nb],
                            w2_t[ki][:ks, m0 : m0 + ms],
                            h1_t[ki][:ks, :nb],
                            start=(ki == 0),
                            stop=(ki == NK2 - 1),
                        )
                        if ki == NK2 - 1:
                            if mi == 2:
                                ht = hfpool.tile([F2R + 1, nb], F16, tag="hf_2")
                                nc.vector.memset(ht[: F2R + 1, :nb], 1.0)
                                nc.vector.tensor_scalar(
                                    ht[:ms, :nb], pts2[mi][:ms, :nb], 0.0, None, op0=Max
                                )
                            else:
                                ht = hfpool.tile([ms, nb], F16, tag=f"hf_{mi}")
                                if mi == 1:
                                    nc.scalar.activation(
                                        ht[:ms, :nb], pts2[mi][:ms, :nb], Relu
                                    )
                                else:
                                    nc.vector.tensor_scalar(
                                        ht[:ms, :nb],
                                        pts2[mi][:ms, :nb],
                                        0.0,
                                        None,
                                        op0=Max,
                                    )
                            hf_t[mi] = ht

            # ---- P8: BM44 K0..K2 ----
            if bm:
                for s in range(4):
                    o = ptbm[:, 4 * F1R + s * F2R : 4 * F1R + (s + 1) * F2R]
                    c0 = s * P
                    for ki in range(3):
                        nc.tensor.matmul(
                            o,
                            h1_t[ki][:, c0 : c0 + P],
                            w2_t[ki][:, 2 * P : H2],
                            start=False,
                            stop=False,
                            skip_group_check=True,
                        )

            # ---- P9: L3 of the previous chunk, tanh, out DMA ----
            if prev is not None:
                emit_l3(prev)

            if bm:
                # ---- P10: L2 K3 round (augmented h1rem: adds b2) ----
                hf_t = []
                ks = KS2[3]
                for mi, (m0, ms) in enumerate(m2fm):
                    nc.tensor.matmul(
                        pts2[mi][:ms, :nb],
                        w2_t[3][:ks, m0 : m0 + ms],
                        h1_t[3][:ks, :nb],
                        start=False,
                        stop=True,
                    )
                    ht = hfpool.tile([ms, nb], F16, tag=f"hf_{mi}")
                    nc.vector.tensor_scalar(
                        ht[:ms, :nb], pts2[mi][:ms, :nb], 0.0, None, op0=Max
                    )
                    hf_t.append(ht)
                # ---- P11: BM44 K3 (closes the psbm group) ----
                for s in range(4):
                    o = ptbm[:, 4 * F1R + s * F2R : 4 * F1R + (s + 1) * F2R]
                    c0 = s * P
                    nc.tensor.matmul(
                        o,
                        h1_t[3][: KS2[3], c0 : c0 + P],
                        w2_t[3][: KS2[3], 2 * P : H2],
                        start=False,
                        stop=(s == 3),
                        skip_group_check=True,
                    )
                # hf_t[2] (augmented hf44T) is filled by the NEXT chunk's
                # combined transpose
                hf44 = hfpool.tile([F2R + 1, 512], F16, tag="hf44")
                hf_t.append(hf44)

            return {
                "hf_t": hf_t,
                "b0": b0,
                "nb": nb,
                "ns": ns,
                "ptbm": ptbm,
                "w3s": w3s,
            }

        def emit_l3(st, flush=False):
            hf_t, b0, nb, ns = st["hf_t"], st["b0"], st["nb"], st["ns"]
            w3c = st["w3s"]
            pb = nb if ns == 1 else P  # partitions live in the last slice
            if flush:
                # the drain chunk's L3 borrows a ps2 bank (already evicted)
                # instead of waiting for the previous tanh's ps3 read
                pt3 = ps2.tile([P, 512], F32, tag="ps2", name="ps3_flush")
            else:
                pt3 = ps3.tile([P, 4 * A], F32, tag="ps3")
            # ki-outer: the hf44T-dependent (ki=2) passes come last so the
            # transpose/copy chain never stalls the earlier passes
            n_mm = ns * NK3
            i = 0
            for ki in range(NK3):
                ks = KS3[ki]
                for s in range(ns):
                    c0, c1 = s * P, min((s + 1) * P, nb)
                    o = pt3[: c1 - c0, s * A : s * A + A]
                    nc.tensor.matmul(
                        o,
                        hf_t[ki][:ks, c0:c1],
                        w3c[ki][:ks, :A],
                        start=(i == 0),
                        stop=(i == n_mm - 1),
                        skip_group_check=True,
                    )
                    i += 1
            ot = opool.tile([P, 4, A], F32, tag="o")
            nc.scalar.activation(ot[:pb, :ns, :], pt3[:pb, : ns * A], Tanh)
            s0 = b0 // P
            # SP ring: x prefetches leave it nearly idle, and the drain's
            # tanh/eviction chain never waits behind a DMA on ACT
            nc.sync.dma_start(out=out[:pb, s0 : s0 + ns, :], in_=ot[:pb, :ns, :])

        def emit_all():
            prev = None
            for ci, (b0, nb) in enumerate(bchunks):
                prev = emit_chunk(ci, b0, nb, prev)
            emit_l3(prev, flush=True)

        if reps > 1:
            with tc.For_i(0, reps, 1):
                emit_all()
        else:
            emit_all()
    return _legalize_wait_counts(nc) if legalize else nc


def _get_nc(BM):
    key = (BM, _plan.get(BM, 0))
    if key not in _nc_cache:
        _nc_cache[key] = _build(BM)
    return _nc_cache[key]


def pack_k(mat, nk):
    # [K, N] -> zero-pad K to nk*128 -> [128, nk, N] with row j*128+p of the
    # original at [p, j, :] (zero rows contribute nothing to the contraction)
    kk, nn = mat.shape
    pad = np.zeros((nk * P, nn), np.float16)
    pad[:kk] = mat.astype(np.float16)
    return np.ascontiguousarray(pad.reshape(nk, P, nn).transpose(1, 0, 2))


def _aug(mat, row):
    # append a bias row to the contraction dim
    return np.concatenate([mat, row.reshape(1, -1)], axis=0)


def kernel(state, idx, W1, b1, W2, b2, W3, b3):
    global last_run
    state = np.asarray(state, dtype=np.float32)
    idx = np.asarray(idx)
    W1 = np.asarray(W1, dtype=np.float32)
    b1 = np.asarray(b1, dtype=np.float32)
    W2 = np.asarray(W2, dtype=np.float32)
    b2 = np.asarray(b2, dtype=np.float32)
    W3 = np.asarray(W3, dtype=np.float32)
    b3 = np.asarray(b3, dtype=np.float32)
    B = state.shape[0]

    # Host-side routing: idx is sorted in the reference workload; fall back to
    # a stable argsort if not, so grouping stays correct for any input.
    idx_i = idx.astype(np.int64)
    perm = None
    if np.any(np.diff(idx_i) < 0):
        perm = np.argsort(idx_i, kind="stable")
        idx_i = idx_i[perm]
        state = state[perm]
    assert idx_i.min() >= 0 and idx_i.max() < G, "idx out of range [0, G)"
    counts = np.bincount(idx_i, minlength=G)[:G]
    offs = np.concatenate([[0], np.cumsum(counts)])

    BM = max(512, int(-(-counts.max() // P) * P))  # round up to 128 rows

    # Overflow rebalancing plan: hot experts' rows beyond A_CAP go to other
    # cores' B-column donor chunk, shrinking every core's stream from BM to
    # A_CAP + B.
    plan_B = 0
    pieces = []  # (donor_expert, start_row_within_expert, n_rows) per core
    if counts.max() > A_CAP:
        ov = [(g, int(c) - A_CAP) for g, c in enumerate(counts) if c > A_CAP]
        for Bc in (32, 64, 96, 128):
            if sum(-(-o // Bc) for _, o in ov) <= G:
                plan_B = Bc
                break
        if plan_B and A_CAP + plan_B < BM:
            for g, o in ov:
                s = A_CAP
                while s < A_CAP + o:
                    n = min(plan_B, A_CAP + o - s)
                    pieces.append((g, s, n))
                    s += n
        else:
            plan_B = 0
    if plan_B:
        _plan[BM] = plan_B
    else:
        _plan.pop(BM, None)
    nc = _get_nc(BM)
    XW = A_CAP + plan_B if plan_B else BM
    NS = (A_CAP // P + 1) if plan_B else BM // P

    # W2 augmented with the b2 row (the kernel's KS2[-1] = 17 rows cover
    # h1[384:400] + the ones row of h1remT)
    w2p = pack_k(_aug(W2, b2), NK2)
    identity = np.eye(P, dtype=np.float16)

    w1p = [pack_k(_aug(W1[g], b1[g]), NK1) for g in range(G)]
    w3p = [pack_k(_aug(W3[g], b3[g]), NK3) for g in range(G)]
    in_maps = []
    for g in range(G):
        nown = min(int(counts[g]), A_CAP) if plan_B else int(counts[g])
        seg = state[offs[g] : offs[g] + nown]
        xg = np.zeros((D + 1, XW), np.float32)
        xg[:D, : seg.shape[0]] = seg.T
        xg[D, :] = 1.0  # ones row -> b1 via W1's augmented row
        m = {
            "xP": None,
            "w1": w1p[g],
            "w2": w2p,
            "w3": w3p[g],
            "ident": identity,
        }
        if plan_B:
            d = g  # donor defaults to self (empty piece)
            if g < len(pieces):
                d, s0r, n = pieces[g]
                prows = state[offs[d] + s0r : offs[d] + s0r + n]
                xg[:D, A_CAP : A_CAP + n] = prows.T
            m["w1b"] = w1p[d]
            m["w3b"] = w3p[d]
        m["xP"] = pack_k(xg, NK1)
        in_maps.append(m)

    globals()["_last_in_maps"] = in_maps
    try:
        last_run = run_bass_kernel_spmd(nc, in_maps, list(range(NCORES)))
    except ModuleNotFoundError:
        # BASS_TRACE set in an env without the axon NTFF hook: retry untraced
        import os

        os.environ["BASS_NEVER_TRACE"] = "1"
        last_run = run_bass_kernel_spmd(nc, in_maps, list(range(NCORES)))

    out = np.empty((B, A), np.float32)
    for g in range(G):
        og = np.asarray(last_run.results[g]["out"])  # [P, NS, A]
        rows = og.transpose(1, 0, 2).reshape(NS * P, A)
        nown = min(int(counts[g]), A_CAP) if plan_B else int(counts[g])
        out[offs[g] : offs[g] + nown] = rows[:nown]
        if plan_B and g < len(pieces):
            d, s0r, n = pieces[g]
            out[offs[d] + s0r : offs[d] + s0r + n] = rows[A_CAP : A_CAP + n]
    if perm is not None:
        inv = np.empty_like(perm)
        inv[perm] = np.arange(B)
        out = out[inv]
    return out


# revision 30
# speedup vs baseline: 1.0288x; 1.0288x over previous
"""Trainium2 Bass kernel for the multi-task ActorNetwork (moe_routing).

Architecture (reference): per-sample expert routing over G=8 tasks:
    h1 = relu(x @ W1[idx] + b1[idx])     x:[B,376]  W1:[8,376,400]
    hf = relu(h1 @ W2 + b2)              W2:[400,300]
    a  = tanh(hf @ W3[idx] + b3[idx])    W3:[8,300,17]

Strategy: idx is sorted, and G == n_cores == 8, so we route on the HOST:
core g receives exactly the contiguous rows with idx == g (zero-padded to a
common BM), plus only ITS expert weights. Each core then runs a dense 3-layer
MLP -- no device-side routing, no collectives, and none of the 8x dense
compute the reference does.

Numerics: fp16 operands with fp32 PSUM accumulation; measured end-to-end
max-abs error vs the fp32 reference ~5e-3 on unit-scale outputs.

Matmul cost on the PE is (output free size) x (cycles/row), so the layout is
chosen to minimize streamed output elements:
  * L1/L2 main tiles (128-feature groups) run feature-major: the contraction
    dim sits on SBUF partitions and the 512-sample chunk streams as the
    moving dim.
  * The ragged remainders (h1's last 16 features, hf's last 44) run
    BATCH-major: out[b_slice, F] = x_sliceT.T @ W[:, rem] streams only F
    elements per pass; one PE transpose per 128-sample slice (through fp16
    PSUM) restores the feature-major layout the next layer needs.  The
    current chunk's 16-col block and the PREVIOUS chunk's 44-col block are
    relu'd into one [128, 4, 109] SBUF tile and share the same transpose
    (the hf44 block sits at column 64 because 45-partition engine reads must
    start at partition 0 or 64).
  * L3 is fully batch-major (17 outputs): lhsT = a 128-column slice of hfT,
    rhs = W3 -- ~17 cycles per slice-pass instead of 512 per K-pass.  All L3
    matmuls of a chunk form ONE PSUM accumulation group in one bank writing
    disjoint 17-column slices.
  * ALL biases ride spare contraction rows: x carries a ones-row at 376 and
    W1's K-slab carries b1 there; the transposed remainders carry a ones-row
    (memset column of the [128,4,62] tile) matched by b2/b3 rows in the
    augmented W2/W3 K-slabs.  No bias operands, no rank-1 bias matmuls.

The last two chunks run the plain feature-major path (K-outer over three L2
groups, borrowing the then-idle psbm bank): slightly more PE work, but the
drain has no transpose/copy chain to serialize on, which keeps the tail to
tanh -> out-DMA -> barrier (~2.8us, dominated by the modeled DMA latency).

Engine split: PE matmuls+transposes; ACT: L1 relu x2, BM16 relu, hf44T copy,
L3 tanh; DVE: L1 relu x1, L2 relu x2, BM44 relu, h1remT copy; Pool: weight
DMAs; SP: x-chunk streaming + out DMAs.  A dummy activation at t~0.5us
preloads the ACT function table off the critical path.
"""

import sys

if "/opt/trn_rl_repo" not in sys.path:
    sys.path.insert(0, "/opt/trn_rl_repo")

from contextlib import ExitStack

import numpy as np

import concourse.bass as bass
import concourse.mybir as mybir
from concourse.bass_utils import run_bass_kernel_spmd
from concourse.tile import TileContext

D, G, H1, H2, A = 376, 8, 400, 300, 17
P = 128
NCORES = 8
F16 = mybir.dt.float16
F32 = mybir.dt.float32

F1R = H1 - 3 * P  # 16: L1 feature remainder
F2R = H2 - 2 * P  # 44: L2 feature remainder
HF44_OFF = 64  # hf44 block column offset (45-partition reads must start at 0/64)
BMC = HF44_OFF + F2R + 1  # 109: combined-transpose tile columns

NK1, NK2, NK3 = 3, 4, 3
KS1 = [128, 128, D - 256]  # x contraction slabs (120-slab holds the ones row)
KS2 = [128, 128, 128, F1R + 1]  # L2 slabs; last = h1rem + ones(b2) row
KS3 = [128, 128, F2R + 1]  # L3 slabs; last = hf44 + ones(b3) row
M1 = [(0, 128), (128, 128), (256, 128), (384, F1R)]
M2 = [(0, 128), (128, 128), (256, F2R)]

_nc_cache = {}
last_run = None  # BassKernelResults of the most recent launch (for profiling)
_last_in_maps = None  # per-core input dicts of the most recent launch

# Cross-core overflow rebalancing: when some experts exceed A_CAP=4096 rows,
# every core runs 8 full own-expert chunks plus one small B-column chunk that
# applies a DONOR expert's weight set; the host routes hot experts' overflow
# rows into other cores' B-chunks.  _plan maps the nominal BM (what the test
# harness computes from counts) to the B width so _build stays reproducible
# from (BM,) alone.
A_CAP = 8 * 512
_plan = {}

_nop_counter = [0]


def _chunks(total, step):
    return [(o, min(step, total - o)) for o in range(0, total, step)]


def _legalize_wait_counts(nc):
    """This container's walrus encodes at most ONE sync-wait per instruction
    (DMA pseudo-instructions especially). Tile freely emits several. Sequencers
    are in-order, so hoisting the surplus waits onto same-engine NoOps placed
    immediately before the instruction is semantics-preserving."""
    for fn in nc.m.functions:
        for bb in fn.blocks:
            insts = list(bb.instructions)
            out = []
            changed = False
            for inst in insts:
                si = inst.sync_info
                waits = list(si.on_wait) if si is not None and si.on_wait else []
                if len(waits) > 1:
                    changed = True
                    for w in waits[:-1]:
                        _nop_counter[0] += 1
                        nop = mybir.InstNoOp(
                            name=f"waitsplit_nop_{_nop_counter[0]}",
                            engine=inst.engine,
                            ins=[],
                            outs=[],
                            sync_info=mybir.SyncInfo(on_wait=[w], on_update=[]),
                        )
                        out.append(nop)
                    si.on_wait = waits[-1:]
                out.append(inst)
            if changed:
                bb.instructions = out
    return nc


def _build(BM, legalize=True, reps=1):
    """Bass program for one core: dense 3-layer MLP over BM rows.

    reps>1 wraps the body in a hardware For_i loop (benchmarking only)."""
    B = _plan.get(BM, 0)
    if B:
        XW = A_CAP + B  # columns of xP; the last chunk is the B donor chunk
        bchunks = _chunks(A_CAP, 512) + [(A_CAP, B)]
        NS = A_CAP // P + 1
    else:
        XW = BM
        bchunks = _chunks(BM, 512)
        NS = BM // P
    nc = bass.Bass()
    xP = nc.declare_dram_parameter("xP", [P, NK1, XW], F16, isOutput=False)
    w1 = nc.declare_dram_parameter("w1", [P, NK1, H1], F16, isOutput=False)
    w2 = nc.declare_dram_parameter("w2", [P, NK2, H2], F16, isOutput=False)
    w3 = nc.declare_dram_parameter("w3", [P, NK3, A], F16, isOutput=False)
    if B:
        w1b = nc.declare_dram_parameter("w1b", [P, NK1, H1], F16, isOutput=False)
        w3b = nc.declare_dram_parameter("w3b", [P, NK3, A], F16, isOutput=False)
    ident = nc.declare_dram_parameter("ident", [P, P], F16, isOutput=False)
    # out[p, s, a] = action a of sample s*128 + p (host re-interleaves)
    out = nc.declare_dram_parameter("out", [P, NS, A], F32, isOutput=True)

    Relu = mybir.ActivationFunctionType.Relu
    Tanh = mybir.ActivationFunctionType.Tanh
    Add = mybir.AluOpType.add
    Max = mybir.AluOpType.max

    with TileContext(nc) as tc, ExitStack() as ctx:
        wpool = ctx.enter_context(tc.tile_pool(name="w", bufs=1))
        xpool = ctx.enter_context(tc.tile_pool(name="x", bufs=3))
        h1pool = ctx.enter_context(tc.tile_pool(name="h1", bufs=3))
        hfpool = ctx.enter_context(tc.tile_pool(name="hf", bufs=3))
        bmpool = ctx.enter_context(tc.tile_pool(name="bm", bufs=2))
        opool = ctx.enter_context(tc.tile_pool(name="o", bufs=3))
        ps1 = ctx.enter_context(tc.tile_pool(name="ps1", bufs=3, space="PSUM"))
        ps2 = ctx.enter_context(tc.tile_pool(name="ps2", bufs=2, space="PSUM"))
        psbm = ctx.enter_context(tc.tile_pool(name="psbm", bufs=1, space="PSUM"))
        pst = ctx.enter_context(tc.tile_pool(name="pst", bufs=1, space="PSUM"))
        ps3 = ctx.enter_context(tc.tile_pool(name="ps3", bufs=1, space="PSUM"))

        def load_weights(param, nk, ncols, name, eng):
            tiles = []
            for ki in range(nk):
                t = wpool.tile([P, ncols], F16, tag=f"{name}_{ki}")
                eng.dma_start(out=t[:, :], in_=param[:, ki, :])
                tiles.append(t)
            return tiles

        # Ring latency tuning: w1's K0 slab rides the ACT HWDGE ring (sem at
        # ~2.4us, the earliest possible) so the very first matmul is not
        # gated on the slower Pool ring; everything else streams on Pool in
        # deadline order, leaving ACT free for chunk-0's evictions.
        w1_t = [None] * NK1
        t0 = wpool.tile([P, H1], F16, tag="w1_0")
        nc.scalar.dma_start(out=t0[:, :], in_=w1[:, 0, :])
        w1_t[0] = t0
        for ki in (1, 2):
            t = wpool.tile([P, H1], F16, tag=f"w1_{ki}", name=f"w1t_{ki}")
            nc.gpsimd.dma_start(out=t[:, :], in_=w1[:, ki, :])
            w1_t[ki] = t
        ident_t = wpool.tile([P, P], F16, tag="ident")
        nc.gpsimd.dma_start(out=ident_t[:, :], in_=ident[:, :])
        w2_t = load_weights(w2, NK2, H2, "w2", nc.gpsimd)
        w3_t = load_weights(w3, NK3, A, "w3", nc.gpsimd)

        # preload the ACT function table off the critical path (the first
        # activation otherwise pays ~1.4us mid-stream)
        seed_t = wpool.tile([1, 1], F32, tag="seed")
        nc.vector.memset(seed_t[:, :], 0.0)
        actw_t = wpool.tile([1, 1], F32, tag="actw")
        nc.scalar.activation(actw_t[:, :], seed_t[:, :], Relu)

        if B:
            w1b_t = load_weights(w1b, NK1, H1, "w1b", nc.gpsimd)
            w3b_t = load_weights(w3b, NK3, A, "w3b", nc.gpsimd)
        else:
            w1b_t, w3b_t = w1_t, w3_t

        def emit_l1_fm(ci, b0, nb, w1s, bm):
            """x DMA + feature-major L1 (+ FM remainder for non-bm chunks)."""
            xt = xpool.tile([P, NK1, 512], F16, tag="x", name=f"xt_{ci}")
            if ci == 0:
                for ki in range(NK1):
                    nc.sync.dma_start(out=xt[:, ki, :nb], in_=xP[:, ki, b0 : b0 + nb])
            else:
                nc.sync.dma_start(out=xt[:, :, :nb], in_=xP[:, :, b0 : b0 + nb])
            nfm1 = 3 if bm else len(M1)
            pts1 = [
                ps1.tile([P, 512], F32, tag="ps1", name=f"ps1_{ci}_{i}")
                for i in range(nfm1)
            ]
            h1_t = [None] * NK2
            order1 = [(ki, mi) for mi in range(nfm1) for ki in range(NK1)]
            for ki, mi in order1:
                m0, ms = M1[mi]
                nc.tensor.matmul(
                    pts1[mi][:ms, :nb],
                    w1s[ki][:, m0 : m0 + ms],
                    xt[:, ki, :nb],
                    start=(ki == 0),
                    stop=(ki == NK1 - 1),
                )
                if ki == NK1 - 1:
                    if mi == 3:
                        # FM remainder (partial chunk): augmented ones row
                        # carries b2 into the next layer's contraction
                        ht = h1pool.tile([F1R + 1, nb], F16, tag="h1_3")
                        # engine ops must start at partition 0: fill the whole
                        # tile with ones, the eviction overwrites rows 0:16
                        nc.vector.memset(ht[: F1R + 1, :nb], 1.0)
                        nc.vector.tensor_scalar(
                            ht[:ms, :nb], pts1[mi][:ms, :nb], 0.0, None, op0=Max
                        )
                    else:
                        ht = h1pool.tile([ms, nb], F16, tag=f"h1_{mi}")
                        on_act = mi < 2 if bm else mi % 2 == 0
                        if on_act:
                            nc.scalar.activation(ht[:ms, :nb], pts1[mi][:ms, :nb], Relu)
                        else:
                            nc.vector.tensor_scalar(
                                ht[:ms, :nb], pts1[mi][:ms, :nb], 0.0, None, op0=Max
                            )
                    h1_t[mi] = ht
            return h1_t, xt

        def emit_chunk(ci, b0, nb, prev, pre_h1=None, inject=None):
            # the last TWO chunks run plain feature-major: slightly more PE
            # work, but the drain has no transpose/copy chain to serialize on
            bm = nb == 512 and ci < len(bchunks) - 2
            ns = (nb + P - 1) // P
            donor = B and ci == len(bchunks) - 1
            w1s = w1b_t if donor else w1_t
            w3s = w3b_t if donor else w3_t

            # ---- P1: x DMA + L1 feature-major ----
            if pre_h1 is not None:
                h1_t, xt = pre_h1
            else:
                h1_t, xt = emit_l1_fm(ci, b0, nb, w1s, bm)
            if inject is not None:
                inject()

            # ---- P2: L1 batch-major remainder (16 features) + the combined
            #      transpose staging tile ----
            ptbm = None
            bmc_sb = bmpool.tile([P, 4, BMC], F16, tag="bmc")
            # ones columns (transpose into the b2/b3 contraction rows of
            # h1remT/hf44T); cheap [128,4] writes, re-set each rotation so
            # CoreSim's fresh-tile NaN canaries never leak into the transpose
            nc.vector.memset(bmc_sb[:, :, F1R : F1R + 1], 1.0)
            nc.vector.memset(bmc_sb[:, :, BMC - 1 : BMC], 1.0)
            if bm:
                # psbm bank: cols [0,4*F1R) = BM16, [4*F1R,..) = BM44; ONE
                # accumulation group from the first BM16 mm to the last BM44
                # mm (each slice's first write lands on pending-zero bytes).
                ptbm = psbm.tile([P, 512], F32, tag="psbm", name=f"ptbm_{ci}")
                for s in range(4):
                    o = ptbm[:, s * F1R : (s + 1) * F1R]
                    c0 = s * P
                    for ki in range(NK1):
                        nc.tensor.matmul(
                            o,
                            xt[:, ki, c0 : c0 + P],
                            w1s[ki][:, 3 * P : H1],
                            start=(s == 0 and ki == 0),
                            stop=False,
                            skip_group_check=True,
                        )
                bm1v = bmc_sb[:, :, :F1R]
                nc.scalar.activation(bm1v, ptbm[:, : 4 * F1R], Relu)

            # ---- P2.5: previous chunk's BM44 relu into the combined tile --
            if prev is not None and prev["ptbm"] is not None:
                nc.vector.tensor_scalar(
                    bmc_sb[:, :, HF44_OFF : HF44_OFF + F2R],
                    prev["ptbm"][:, 4 * F1R : 4 * (F1R + F2R)],
                    0.0,
                    None,
                    op0=Max,
                )

            # ---- P4..P7: L2 feature-major K-rounds 0..2 (+T after K1) ----
            nfm2 = 2 if bm else len(M2)
            m2fm = M2[:nfm2]
            pts2 = [
                ps2.tile([P, 512], F32, tag="ps2", name=f"ps2_{ci}_{i}")
                for i in range(min(nfm2, 2))
            ]
            if nfm2 == 3:
                # FM chunks borrow the (idle at that point) psbm bank for the
                # third concurrent K-outer group
                pts2.append(psbm.tile([P, 512], F32, tag="psbm", name=f"psd_{ci}"))

            def l2_round(ki):
                ks = KS2[ki]
                for mi, (m0, ms) in enumerate(m2fm):
                    nc.tensor.matmul(
                        pts2[mi][:ms, :nb],
                        w2_t[ki][:ks, m0 : m0 + ms],
                        h1_t[ki][:ks, :nb],
                        start=(ki == 0),
                        stop=(ki == NK2 - 1),
                    )

            def emit_transposes():
                # one [128,109] transpose per slice: rows 0:17 become the
                # augmented h1remT (this chunk), rows 64:109 the augmented
                # hf44T (previous chunk; 45-partition engine reads must start
                # at partition 0 or 64, hence the column gap)
                ptt = pst.tile([BMC, 512], F16, tag="pst")
                for s in range(4):
                    nc.tensor.transpose(
                        ptt[:BMC, s * P : (s + 1) * P],
                        bmc_sb[:, s, :],
                        ident_t[:, :],
                    )
                if prev is not None and prev["ptbm"] is not None:
                    # ACT as copy engine: values are post-relu/ones (>=0)
                    nc.scalar.activation(
                        prev["hf_t"][2][: F2R + 1, :],
                        ptt[HF44_OFF : HF44_OFF + F2R + 1, :],
                        Relu,
                    )
                if bm:
                    h1r = h1pool.tile([F1R + 1, 512], F16, tag="h1r")
                    nc.vector.tensor_scalar(
                        h1r[:, :], ptt[: F1R + 1, :], 0.0, None, op0=Add
                    )
                    h1_t[3] = h1r

            if bm:
                l2_round(0)
                l2_round(1)
                emit_transposes()
                l2_round(2)
            else:
                emit_transposes()
                # drain chunks: the FM-512 chunk runs M-outer so each hf tile
                # stops (and evicts) a third of the chunk early, spreading the
                # big [128,512] evictions instead of bunching them at the end;
                # the tiny B chunk keeps K-outer
                hf_t = [None] * len(M2)
                if nb == 512:
                    order2 = [(ki, mi) for mi in range(len(m2fm)) for ki in range(NK2)]
                else:
                    order2 = [(ki, mi) for ki in range(NK2) for mi in range(len(m2fm))]
                for ki, mi in order2:
                    ks = KS2[ki]
                    if True:
                        m0, ms = m2fm[mi]
                        nc.tensor.matmul(
                            pts2[mi][:ms, :nb],
                            w2_t[ki][:ks, m0 : m0 + ms],
                            h1_t[ki][:ks, :nb],
                            start=(ki == 0),
                            stop=(ki == NK2 - 1),
                        )
                        if ki == NK2 - 1:
                            if mi == 2:
                                ht = hfpool.tile([F2R + 1, nb], F16, tag="hf_2")
                                nc.vector.memset(ht[: F2R + 1, :nb], 1.0)
                                nc.vector.tensor_scalar(
                                    ht[:ms, :nb], pts2[mi][:ms, :nb], 0.0, None, op0=Max
                                )
                            else:
                                ht = hfpool.tile([ms, nb], F16, tag=f"hf_{mi}")
                                if mi == 1:
                                    nc.scalar.activation(
                                        ht[:ms, :nb], pts2[mi][:ms, :nb], Relu
                                    )
                                else:
                                    nc.vector.tensor_scalar(
                                        ht[:ms, :nb],
                                        pts2[mi][:ms, :nb],
                                        0.0,
                                        None,
                                        op0=Max,
                                    )
                            hf_t[mi] = ht

            # ---- P8: BM44 K0..K2 ----
            if bm:
                for s in range(4):
                    o = ptbm[:, 4 * F1R + s * F2R : 4 * F1R + (s + 1) * F2R]
                    c0 = s * P
                    for ki in range(3):
                        nc.tensor.matmul(
                            o,
                            h1_t[ki][:, c0 : c0 + P],
                            w2_t[ki][:, 2 * P : H2],
                            start=False,
                            stop=False,
                            skip_group_check=True,
                        )

            # ---- P9: L3 of the previous chunk, tanh, out DMA ----
            if prev is not None:
                emit_l3(prev)

            if bm:
                # ---- P10: L2 K3 round (augmented h1rem: adds b2) ----
                hf_t = []
                ks = KS2[3]
                for mi, (m0, ms) in enumerate(m2fm):
                    nc.tensor.matmul(
                        pts2[mi][:ms, :nb],
                        w2_t[3][:ks, m0 : m0 + ms],
                        h1_t[3][:ks, :nb],
                        start=False,
                        stop=True,
                    )
                    ht = hfpool.tile([ms, nb], F16, tag=f"hf_{mi}")
                    nc.vector.tensor_scalar(
                        ht[:ms, :nb], pts2[mi][:ms, :nb], 0.0, None, op0=Max
                    )
                    hf_t.append(ht)
                # ---- P11: BM44 K3 (closes the psbm group) ----
                for s in range(4):
                    o = ptbm[:, 4 * F1R + s * F2R : 4 * F1R + (s + 1) * F2R]
                    c0 = s * P
                    nc.tensor.matmul(
                        o,
                        h1_t[3][: KS2[3], c0 : c0 + P],
                        w2_t[3][: KS2[3], 2 * P : H2],
                        start=False,
                        stop=(s == 3),
                        skip_group_check=True,
                    )
                # hf_t[2] (augmented hf44T) is filled by the NEXT chunk's
                # combined transpose
                hf44 = hfpool.tile([F2R + 1, 512], F16, tag="hf44")
                hf_t.append(hf44)

            return {
                "hf_t": hf_t,
                "b0": b0,
                "nb": nb,
                "ns": ns,
                "ptbm": ptbm,
                "w3s": w3s,
            }

        def emit_l3(st, flush=False):
            hf_t, b0, nb, ns = st["hf_t"], st["b0"], st["nb"], st["ns"]
            w3c = st["w3s"]
            pb = nb if ns == 1 else P  # partitions live in the last slice
            if flush:
                # the drain chunk's L3 borrows a ps2 bank (already evicted)
                # instead of waiting for the previous tanh's ps3 read
                pt3 = ps2.tile([P, 512], F32, tag="ps2", name="ps3_flush")
            else:
                pt3 = ps3.tile([P, 4 * A], F32, tag="ps3")
            # ki-outer: the hf44T-dependent (ki=2) passes come last so the
            # transpose/copy chain never stalls the earlier passes
            n_mm = ns * NK3
            i = 0
            for ki in range(NK3):
                ks = KS3[ki]
                for s in range(ns):
                    c0, c1 = s * P, min((s + 1) * P, nb)
                    o = pt3[: c1 - c0, s * A : s * A + A]
                    nc.tensor.matmul(
                        o,
                        hf_t[ki][:ks, c0:c1],
                        w3c[ki][:ks, :A],
                        start=(i == 0),
                        stop=(i == n_mm - 1),
                        skip_group_check=True,
                    )
                    i += 1
            ot = opool.tile([P, 4, A], F32, tag="o")
            nc.scalar.activation(ot[:pb, :ns, :], pt3[:pb, : ns * A], Tanh)
            s0 = b0 // P
            # SP ring: x prefetches leave it nearly idle, and the drain's
            # tanh/eviction chain never waits behind a DMA on ACT
            nc.sync.dma_start(out=out[:pb, s0 : s0 + ns, :], in_=ot[:pb, :ns, :])

        def emit_all():
            prev = None
            n = len(bchunks)
            box = {}
            for ci, (b0, nb) in enumerate(bchunks):
                if B and ci == n - 1:
                    prev = emit_chunk(ci, b0, nb, prev, pre_h1=box.get("b"))
                elif B and ci == n - 2:
                    # emit the B donor chunk's L1 inside this chunk's stream:
                    # its eviction ping-pong overlaps real work instead of
                    # serializing in the drain
                    def inj():
                        bb0, bnb = bchunks[n - 1]
                        box["b"] = emit_l1_fm(n - 1, bb0, bnb, w1b_t, False)

                    prev = emit_chunk(ci, b0, nb, prev, inject=inj)
                else:
                    prev = emit_chunk(ci, b0, nb, prev)
            emit_l3(prev, flush=True)

        if reps > 1:
            with tc.For_i(0, reps, 1):
                emit_all()
        else:
            emit_all()
    return _legalize_wait_counts(nc) if legalize else nc


def _get_nc(BM):
    key = (BM, _plan.get(BM, 0))
    if key not in _nc_cache:
        _nc_cache[key] = _build(BM)
    return _nc_cache[key]


def pack_k(mat, nk):
    # [K, N] -> zero-pad K to nk*128 -> [128, nk, N] with row j*128+p of the
    # original at [p, j, :] (zero rows contribute nothing to the contraction)
    kk, nn = mat.shape
    pad = np.zeros((nk * P, nn), np.float16)
    pad[:kk] = mat.astype(np.float16)
    return np.ascontiguousarray(pad.reshape(nk, P, nn).transpose(1, 0, 2))


def _aug(mat, row):
    # append a bias row to the contraction dim
    return np.concatenate([mat, row.reshape(1, -1)], axis=0)


def kernel(state, idx, W1, b1, W2, b2, W3, b3):
    global last_run
    state = np.asarray(state, dtype=np.float32)
    idx = np.asarray(idx)
    W1 = np.asarray(W1, dtype=np.float32)
    b1 = np.asarray(b1, dtype=np.float32)
    W2 = np.asarray(W2, dtype=np.float32)
    b2 = np.asarray(b2, dtype=np.float32)
    W3 = np.asarray(W3, dtype=np.float32)
    b3 = np.asarray(b3, dtype=np.float32)
    B = state.shape[0]

    # Host-side routing: idx is sorted in the reference workload; fall back to
    # a stable argsort if not, so grouping stays correct for any input.
    idx_i = idx.astype(np.int64)
    perm = None
    if np.any(np.diff(idx_i) < 0):
        perm = np.argsort(idx_i, kind="stable")
        idx_i = idx_i[perm]
        state = state[perm]
    assert idx_i.min() >= 0 and idx_i.max() < G, "idx out of range [0, G)"
    counts = np.bincount(idx_i, minlength=G)[:G]
    offs = np.concatenate([[0], np.cumsum(counts)])

    BM = max(512, int(-(-counts.max() // P) * P))  # round up to 128 rows

    # Overflow rebalancing plan: hot experts' rows beyond A_CAP go to other
    # cores' B-column donor chunk, shrinking every core's stream from BM to
    # A_CAP + B.
    plan_B = 0
    pieces = []  # (donor_expert, start_row_within_expert, n_rows) per core
    if counts.max() > A_CAP:
        ov = [(g, int(c) - A_CAP) for g, c in enumerate(counts) if c > A_CAP]
        for Bc in (32, 64, 96, 128):
            if sum(-(-o // Bc) for _, o in ov) <= G:
                plan_B = Bc
                break
        if plan_B and A_CAP + plan_B < BM:
            for g, o in ov:
                s = A_CAP
                while s < A_CAP + o:
                    n = min(plan_B, A_CAP + o - s)
                    pieces.append((g, s, n))
                    s += n
        else:
            plan_B = 0
    if plan_B:
        _plan[BM] = plan_B
    else:
        _plan.pop(BM, None)
    nc = _get_nc(BM)
    XW = A_CAP + plan_B if plan_B else BM
    NS = (A_CAP // P + 1) if plan_B else BM // P

    # W2 augmented with the b2 row (the kernel's KS2[-1] = 17 rows cover
    # h1[384:400] + the ones row of h1remT)
    w2p = pack_k(_aug(W2, b2), NK2)
    identity = np.eye(P, dtype=np.float16)

    w1p = [pack_k(_aug(W1[g], b1[g]), NK1) for g in range(G)]
    w3p = [pack_k(_aug(W3[g], b3[g]), NK3) for g in range(G)]
    in_maps = []
    for g in range(G):
        nown = min(int(counts[g]), A_CAP) if plan_B else int(counts[g])
        seg = state[offs[g] : offs[g] + nown]
        xg = np.zeros((D + 1, XW), np.float32)
        xg[:D, : seg.shape[0]] = seg.T
        xg[D, :] = 1.0  # ones row -> b1 via W1's augmented row
        m = {
            "xP": None,
            "w1": w1p[g],
            "w2": w2p,
            "w3": w3p[g],
            "ident": identity,
        }
        if plan_B:
            d = g  # donor defaults to self (empty piece)
            if g < len(pieces):
                d, s0r, n = pieces[g]
                prows = state[offs[d] + s0r : offs[d] + s0r + n]
                xg[:D, A_CAP : A_CAP + n] = prows.T
            m["w1b"] = w1p[d]
            m["w3b"] = w3p[d]
        m["xP"] = pack_k(xg, NK1)
        in_maps.append(m)

    globals()["_last_in_maps"] = in_maps
    try:
        last_run = run_bass_kernel_spmd(nc, in_maps, list(range(NCORES)))
    except ModuleNotFoundError:
        # BASS_TRACE set in an env without the axon NTFF hook: retry untraced
        import os

        os.environ["BASS_NEVER_TRACE"] = "1"
        last_run = run_bass_kernel_spmd(nc, in_maps, list(range(NCORES)))

    out = np.empty((B, A), np.float32)
    for g in range(G):
        og = np.asarray(last_run.results[g]["out"])  # [P, NS, A]
        rows = og.transpose(1, 0, 2).reshape(NS * P, A)
        nown = min(int(counts[g]), A_CAP) if plan_B else int(counts[g])
        out[offs[g] : offs[g] + nown] = rows[:nown]
        if plan_B and g < len(pieces):
            d, s0r, n = pieces[g]
            out[offs[d] + s0r : offs[d] + s0r + n] = rows[A_CAP : A_CAP + n]
    if perm is not None:
        inv = np.empty_like(perm)
        inv[perm] = np.arange(B)
        out = out[inv]
    return out


# revision 31
# speedup vs baseline: 1.0298x; 1.0009x over previous
"""Trainium2 Bass kernel for the multi-task ActorNetwork (moe_routing).

Architecture (reference): per-sample expert routing over G=8 tasks:
    h1 = relu(x @ W1[idx] + b1[idx])     x:[B,376]  W1:[8,376,400]
    hf = relu(h1 @ W2 + b2)              W2:[400,300]
    a  = tanh(hf @ W3[idx] + b3[idx])    W3:[8,300,17]

Strategy: idx is sorted, and G == n_cores == 8, so we route on the HOST:
core g receives exactly the contiguous rows with idx == g (zero-padded to a
common BM), plus only ITS expert weights. Each core then runs a dense 3-layer
MLP -- no device-side routing, no collectives, and none of the 8x dense
compute the reference does.

Numerics: fp16 operands with fp32 PSUM accumulation; measured end-to-end
max-abs error vs the fp32 reference ~5e-3 on unit-scale outputs.

Matmul cost on the PE is (output free size) x (cycles/row), so the layout is
chosen to minimize streamed output elements:
  * L1/L2 main tiles (128-feature groups) run feature-major: the contraction
    dim sits on SBUF partitions and the 512-sample chunk streams as the
    moving dim.
  * The ragged remainders (h1's last 16 features, hf's last 44) run
    BATCH-major: out[b_slice, F] = x_sliceT.T @ W[:, rem] streams only F
    elements per pass; one PE transpose per 128-sample slice (through fp16
    PSUM) restores the feature-major layout the next layer needs.  The
    current chunk's 16-col block and the PREVIOUS chunk's 44-col block are
    relu'd into one [128, 4, 109] SBUF tile and share the same transpose
    (the hf44 block sits at column 64 because 45-partition engine reads must
    start at partition 0 or 64).
  * L3 is fully batch-major (17 outputs): lhsT = a 128-column slice of hfT,
    rhs = W3 -- ~17 cycles per slice-pass instead of 512 per K-pass.  All L3
    matmuls of a chunk form ONE PSUM accumulation group in one bank writing
    disjoint 17-column slices.
  * ALL biases ride spare contraction rows: x carries a ones-row at 376 and
    W1's K-slab carries b1 there; the transposed remainders carry a ones-row
    (memset column of the [128,4,62] tile) matched by b2/b3 rows in the
    augmented W2/W3 K-slabs.  No bias operands, no rank-1 bias matmuls.

The last two chunks run the plain feature-major path (K-outer over three L2
groups, borrowing the then-idle psbm bank): slightly more PE work, but the
drain has no transpose/copy chain to serialize on, which keeps the tail to
tanh -> out-DMA -> barrier (~2.8us, dominated by the modeled DMA latency).

Engine split: PE matmuls+transposes; ACT: L1 relu x2, BM16 relu, hf44T copy,
L3 tanh; DVE: L1 relu x1, L2 relu x2, BM44 relu, h1remT copy; Pool: weight
DMAs; SP: x-chunk streaming + out DMAs.  A dummy activation at t~0.5us
preloads the ACT function table off the critical path.
"""

import sys

if "/opt/trn_rl_repo" not in sys.path:
    sys.path.insert(0, "/opt/trn_rl_repo")

from contextlib import ExitStack

import numpy as np

import concourse.bass as bass
import concourse.mybir as mybir
from concourse.bass_utils import run_bass_kernel_spmd
from concourse.tile import TileContext

D, G, H1, H2, A = 376, 8, 400, 300, 17
P = 128
NCORES = 8
F16 = mybir.dt.float16
F32 = mybir.dt.float32

F1R = H1 - 3 * P  # 16: L1 feature remainder
F2R = H2 - 2 * P  # 44: L2 feature remainder
HF44_OFF = 64  # hf44 block column offset (45-partition reads must start at 0/64)
BMC = HF44_OFF + F2R + 1  # 109: combined-transpose tile columns

NK1, NK2, NK3 = 3, 4, 3
KS1 = [128, 128, D - 256]  # x contraction slabs (120-slab holds the ones row)
KS2 = [128, 128, 128, F1R + 1]  # L2 slabs; last = h1rem + ones(b2) row
KS3 = [128, 128, F2R + 1]  # L3 slabs; last = hf44 + ones(b3) row
M1 = [(0, 128), (128, 128), (256, 128), (384, F1R)]
M2 = [(0, 128), (128, 128), (256, F2R)]

_nc_cache = {}
last_run = None  # BassKernelResults of the most recent launch (for profiling)
_last_in_maps = None  # per-core input dicts of the most recent launch

# Cross-core overflow rebalancing: when some experts exceed A_CAP=4096 rows,
# every core runs 8 full own-expert chunks plus one small B-column chunk that
# applies a DONOR expert's weight set; the host routes hot experts' overflow
# rows into other cores' B-chunks.  _plan maps the nominal BM (what the test
# harness computes from counts) to the B width so _build stays reproducible
# from (BM,) alone.
A_CAP = 8 * 512
_plan = {}

_nop_counter = [0]


def _chunks(total, step):
    return [(o, min(step, total - o)) for o in range(0, total, step)]


def _legalize_wait_counts(nc):
    """This container's walrus encodes at most ONE sync-wait per instruction
    (DMA pseudo-instructions especially). Tile freely emits several. Sequencers
    are in-order, so hoisting the surplus waits onto same-engine NoOps placed
    immediately before the instruction is semantics-preserving."""
    for fn in nc.m.functions:
        for bb in fn.blocks:
            insts = list(bb.instructions)
            out = []
            changed = False
            for inst in insts:
                si = inst.sync_info
                waits = list(si.on_wait) if si is not None and si.on_wait else []
                if len(waits) > 1:
                    changed = True
                    for w in waits[:-1]:
                        _nop_counter[0] += 1
                        nop = mybir.InstNoOp(
                            name=f"waitsplit_nop_{_nop_counter[0]}",
                            engine=inst.engine,
                            ins=[],
                            outs=[],
                            sync_info=mybir.SyncInfo(on_wait=[w], on_update=[]),
                        )
                        out.append(nop)
                    si.on_wait = waits[-1:]
                out.append(inst)
            if changed:
                bb.instructions = out
    return nc


def _build(BM, legalize=True, reps=1):
    """Bass program for one core: dense 3-layer MLP over BM rows.

    reps>1 wraps the body in a hardware For_i loop (benchmarking only)."""
    B = _plan.get(BM, 0)
    if B:
        XW = A_CAP + B  # columns of xP; the last chunk is the B donor chunk
        bchunks = _chunks(A_CAP, 512) + [(A_CAP, B)]
        NS = A_CAP // P + 1
    else:
        XW = BM
        bchunks = _chunks(BM, 512)
        NS = BM // P
    nc = bass.Bass()
    xP = nc.declare_dram_parameter("xP", [P, NK1, XW], F16, isOutput=False)
    w1 = nc.declare_dram_parameter("w1", [P, NK1, H1], F16, isOutput=False)
    w2 = nc.declare_dram_parameter("w2", [P, NK2, H2], F16, isOutput=False)
    w3 = nc.declare_dram_parameter("w3", [P, NK3, A], F16, isOutput=False)
    if B:
        w1b = nc.declare_dram_parameter("w1b", [P, NK1, H1], F16, isOutput=False)
        w3b = nc.declare_dram_parameter("w3b", [P, NK3, A], F16, isOutput=False)
    ident = nc.declare_dram_parameter("ident", [P, P], F16, isOutput=False)
    # out[p, s, a] = action a of sample s*128 + p (host re-interleaves)
    out = nc.declare_dram_parameter("out", [P, NS, A], F32, isOutput=True)

    Relu = mybir.ActivationFunctionType.Relu
    Tanh = mybir.ActivationFunctionType.Tanh
    Add = mybir.AluOpType.add
    Max = mybir.AluOpType.max

    with TileContext(nc) as tc, ExitStack() as ctx:
        wpool = ctx.enter_context(tc.tile_pool(name="w", bufs=1))
        xpool = ctx.enter_context(tc.tile_pool(name="x", bufs=3))
        h1pool = ctx.enter_context(tc.tile_pool(name="h1", bufs=3))
        hfpool = ctx.enter_context(tc.tile_pool(name="hf", bufs=3))
        bmpool = ctx.enter_context(tc.tile_pool(name="bm", bufs=2))
        opool = ctx.enter_context(tc.tile_pool(name="o", bufs=3))
        ps1 = ctx.enter_context(tc.tile_pool(name="ps1", bufs=3, space="PSUM"))
        ps2 = ctx.enter_context(tc.tile_pool(name="ps2", bufs=2, space="PSUM"))
        psbm = ctx.enter_context(tc.tile_pool(name="psbm", bufs=1, space="PSUM"))
        pst = ctx.enter_context(tc.tile_pool(name="pst", bufs=1, space="PSUM"))
        ps3 = ctx.enter_context(tc.tile_pool(name="ps3", bufs=1, space="PSUM"))

        def load_weights(param, nk, ncols, name, eng):
            tiles = []
            for ki in range(nk):
                t = wpool.tile([P, ncols], F16, tag=f"{name}_{ki}")
                eng.dma_start(out=t[:, :], in_=param[:, ki, :])
                tiles.append(t)
            return tiles

        # Ring latency tuning: w1's K0 slab rides the ACT HWDGE ring (sem at
        # ~2.4us, the earliest possible) so the very first matmul is not
        # gated on the slower Pool ring; everything else streams on Pool in
        # deadline order, leaving ACT free for chunk-0's evictions.
        w1_t = [None] * NK1
        t0 = wpool.tile([P, H1], F16, tag="w1_0")
        nc.scalar.dma_start(out=t0[:, :], in_=w1[:, 0, :])
        w1_t[0] = t0
        for ki in (1, 2):
            t = wpool.tile([P, H1], F16, tag=f"w1_{ki}", name=f"w1t_{ki}")
            nc.gpsimd.dma_start(out=t[:, :], in_=w1[:, ki, :])
            w1_t[ki] = t
        ident_t = wpool.tile([P, P], F16, tag="ident")
        nc.gpsimd.dma_start(out=ident_t[:, :], in_=ident[:, :])
        w2_t = load_weights(w2, NK2, H2, "w2", nc.gpsimd)
        w3_t = load_weights(w3, NK3, A, "w3", nc.gpsimd)

        # preload the ACT function table off the critical path (the first
        # activation otherwise pays ~1.4us mid-stream)
        seed_t = wpool.tile([1, 1], F32, tag="seed")
        nc.vector.memset(seed_t[:, :], 0.0)
        actw_t = wpool.tile([1, 1], F32, tag="actw")
        nc.scalar.activation(actw_t[:, :], seed_t[:, :], Relu)

        if B:
            w1b_t = load_weights(w1b, NK1, H1, "w1b", nc.gpsimd)
            w3b_t = load_weights(w3b, NK3, A, "w3b", nc.gpsimd)
        else:
            w1b_t, w3b_t = w1_t, w3_t

        def emit_chunk(ci, b0, nb, prev):
            # the last TWO chunks run plain feature-major: slightly more PE
            # work, but the drain has no transpose/copy chain to serialize on
            bm = nb == 512 and ci < len(bchunks) - 2
            ns = (nb + P - 1) // P
            donor = B and ci == len(bchunks) - 1
            w1s = w1b_t if donor else w1_t
            w3s = w3b_t if donor else w3_t

            # ---- x DMA (chunk 0: per-K-slab so the first passes start early)
            xt = xpool.tile([P, NK1, 512], F16, tag="x")
            if ci == 0:
                for ki in range(NK1):
                    nc.sync.dma_start(out=xt[:, ki, :nb], in_=xP[:, ki, b0 : b0 + nb])
            else:
                nc.sync.dma_start(out=xt[:, :, :nb], in_=xP[:, :, b0 : b0 + nb])

            # ---- P1: L1 feature-major tiles (b1 rides x's ones row) ----
            nfm1 = 3 if bm else len(M1)
            pts1 = [
                ps1.tile([P, 512], F32, tag="ps1", name=f"ps1_{ci}_{i}")
                for i in range(min(nfm1, 3))
            ]
            if not bm and nfm1 == 4:
                # last/partial chunk: M-outer, M3 reuses M0's bank after its
                # eviction (sequential groups -- no WAR deadlock)
                pts1.append(ps1.tile([P, 512], F32, tag="ps1", name=f"ps1_{ci}_3"))
            h1_t = [None] * NK2
            order1 = [(ki, mi) for mi in range(nfm1) for ki in range(NK1)]
            for ki, mi in order1:
                m0, ms = M1[mi]
                nc.tensor.matmul(
                    pts1[mi][:ms, :nb],
                    w1s[ki][:, m0 : m0 + ms],
                    xt[:, ki, :nb],
                    start=(ki == 0),
                    stop=(ki == NK1 - 1),
                )
                if ki == NK1 - 1:
                    if mi == 3:
                        # FM remainder (partial chunk): augmented ones row
                        # carries b2 into the next layer's contraction
                        ht = h1pool.tile([F1R + 1, nb], F16, tag="h1_3")
                        # engine ops must start at partition 0: fill the whole
                        # tile with ones, the eviction overwrites rows 0:16
                        nc.vector.memset(ht[: F1R + 1, :nb], 1.0)
                        nc.vector.tensor_scalar(
                            ht[:ms, :nb], pts1[mi][:ms, :nb], 0.0, None, op0=Max
                        )
                    else:
                        ht = h1pool.tile([ms, nb], F16, tag=f"h1_{mi}")
                        on_act = mi < 2 if bm else mi % 2 == 0
                        if on_act:
                            nc.scalar.activation(ht[:ms, :nb], pts1[mi][:ms, :nb], Relu)
                        else:
                            nc.vector.tensor_scalar(
                                ht[:ms, :nb], pts1[mi][:ms, :nb], 0.0, None, op0=Max
                            )
                    h1_t[mi] = ht

            # ---- P2: L1 batch-major remainder (16 features) + the combined
            #      transpose staging tile ----
            ptbm = None
            bmc_sb = bmpool.tile([P, 4, BMC], F16, tag="bmc")
            # ones columns (transpose into the b2/b3 contraction rows of
            # h1remT/hf44T); cheap [128,4] writes, re-set each rotation so
            # CoreSim's fresh-tile NaN canaries never leak into the transpose
            nc.vector.memset(bmc_sb[:, :, F1R : F1R + 1], 1.0)
            nc.vector.memset(bmc_sb[:, :, BMC - 1 : BMC], 1.0)
            if bm:
                # psbm bank: cols [0,4*F1R) = BM16, [4*F1R,..) = BM44; ONE
                # accumulation group from the first BM16 mm to the last BM44
                # mm (each slice's first write lands on pending-zero bytes).
                ptbm = psbm.tile([P, 512], F32, tag="psbm", name=f"ptbm_{ci}")
                for s in range(4):
                    o = ptbm[:, s * F1R : (s + 1) * F1R]
                    c0 = s * P
                    for ki in range(NK1):
                        nc.tensor.matmul(
                            o,
                            xt[:, ki, c0 : c0 + P],
                            w1s[ki][:, 3 * P : H1],
                            start=(s == 0 and ki == 0),
                            stop=False,
                            skip_group_check=True,
                        )
                bm1v = bmc_sb[:, :, :F1R]
                nc.scalar.activation(bm1v, ptbm[:, : 4 * F1R], Relu)

            # ---- P2.5: previous chunk's BM44 relu into the combined tile --
            if prev is not None and prev["ptbm"] is not None:
                nc.vector.tensor_scalar(
                    bmc_sb[:, :, HF44_OFF : HF44_OFF + F2R],
                    prev["ptbm"][:, 4 * F1R : 4 * (F1R + F2R)],
                    0.0,
                    None,
                    op0=Max,
                )

            # ---- P4..P7: L2 feature-major K-rounds 0..2 (+T after K1) ----
            nfm2 = 2 if bm else len(M2)
            m2fm = M2[:nfm2]
            pts2 = [
                ps2.tile([P, 512], F32, tag="ps2", name=f"ps2_{ci}_{i}")
                for i in range(min(nfm2, 2))
            ]
            if nfm2 == 3:
                # FM chunks borrow the (idle at that point) psbm bank for the
                # third concurrent K-outer group
                pts2.append(psbm.tile([P, 512], F32, tag="psbm", name=f"psd_{ci}"))

            def l2_round(ki):
                ks = KS2[ki]
                for mi, (m0, ms) in enumerate(m2fm):
                    nc.tensor.matmul(
                        pts2[mi][:ms, :nb],
                        w2_t[ki][:ks, m0 : m0 + ms],
                        h1_t[ki][:ks, :nb],
                        start=(ki == 0),
                        stop=(ki == NK2 - 1),
                    )

            def emit_transposes():
                # one [128,109] transpose per slice: rows 0:17 become the
                # augmented h1remT (this chunk), rows 64:109 the augmented
                # hf44T (previous chunk; 45-partition engine reads must start
                # at partition 0 or 64, hence the column gap)
                ptt = pst.tile([BMC, 512], F16, tag="pst")
                for s in range(4):
                    nc.tensor.transpose(
                        ptt[:BMC, s * P : (s + 1) * P],
                        bmc_sb[:, s, :],
                        ident_t[:, :],
                    )
                if prev is not None and prev["ptbm"] is not None:
                    # ACT as copy engine: values are post-relu/ones (>=0)
                    nc.scalar.activation(
                        prev["hf_t"][2][: F2R + 1, :],
                        ptt[HF44_OFF : HF44_OFF + F2R + 1, :],
                        Relu,
                    )
                if bm:
                    h1r = h1pool.tile([F1R + 1, 512], F16, tag="h1r")
                    nc.vector.tensor_scalar(
                        h1r[:, :], ptt[: F1R + 1, :], 0.0, None, op0=Add
                    )
                    h1_t[3] = h1r

            if bm:
                l2_round(0)
                l2_round(1)
                emit_transposes()
                l2_round(2)
            else:
                emit_transposes()
                # drain chunks: the FM-512 chunk runs M-outer so each hf tile
                # stops (and evicts) a third of the chunk early, spreading the
                # big [128,512] evictions instead of bunching them at the end;
                # the tiny B chunk keeps K-outer
                hf_t = [None] * len(M2)
                if nb == 512:
                    order2 = [(ki, mi) for mi in range(len(m2fm)) for ki in range(NK2)]
                else:
                    order2 = [(ki, mi) for ki in range(NK2) for mi in range(len(m2fm))]
                for ki, mi in order2:
                    ks = KS2[ki]
                    if True:
                        m0, ms = m2fm[mi]
                        nc.tensor.matmul(
                            pts2[mi][:ms, :nb],
                            w2_t[ki][:ks, m0 : m0 + ms],
                            h1_t[ki][:ks, :nb],
                            start=(ki == 0),
                            stop=(ki == NK2 - 1),
                        )
                        if ki == NK2 - 1:
                            if mi == 2:
                                ht = hfpool.tile([F2R + 1, nb], F16, tag="hf_2")
                                nc.vector.memset(ht[: F2R + 1, :nb], 1.0)
                                nc.vector.tensor_scalar(
                                    ht[:ms, :nb], pts2[mi][:ms, :nb], 0.0, None, op0=Max
                                )
                            else:
                                ht = hfpool.tile([ms, nb], F16, tag=f"hf_{mi}")
                                if mi == 1:
                                    nc.scalar.activation(
                                        ht[:ms, :nb], pts2[mi][:ms, :nb], Relu
                                    )
                                else:
                                    nc.vector.tensor_scalar(
                                        ht[:ms, :nb],
                                        pts2[mi][:ms, :nb],
                                        0.0,
                                        None,
                                        op0=Max,
                                    )
                            hf_t[mi] = ht

            # ---- P8: BM44 K0..K2 ----
            if bm:
                for s in range(4):
                    o = ptbm[:, 4 * F1R + s * F2R : 4 * F1R + (s + 1) * F2R]
                    c0 = s * P
                    for ki in range(3):
                        nc.tensor.matmul(
                            o,
                            h1_t[ki][:, c0 : c0 + P],
                            w2_t[ki][:, 2 * P : H2],
                            start=False,
                            stop=False,
                            skip_group_check=True,
                        )

            # ---- P9: L3 of the previous chunk, tanh, out DMA ----
            if prev is not None:
                emit_l3(prev)

            if bm:
                # ---- P10: L2 K3 round (augmented h1rem: adds b2) ----
                hf_t = []
                ks = KS2[3]
                for mi, (m0, ms) in enumerate(m2fm):
                    nc.tensor.matmul(
                        pts2[mi][:ms, :nb],
                        w2_t[3][:ks, m0 : m0 + ms],
                        h1_t[3][:ks, :nb],
                        start=False,
                        stop=True,
                    )
                    ht = hfpool.tile([ms, nb], F16, tag=f"hf_{mi}")
                    nc.vector.tensor_scalar(
                        ht[:ms, :nb], pts2[mi][:ms, :nb], 0.0, None, op0=Max
                    )
                    hf_t.append(ht)
                # ---- P11: BM44 K3 (closes the psbm group) ----
                for s in range(4):
                    o = ptbm[:, 4 * F1R + s * F2R : 4 * F1R + (s + 1) * F2R]
                    c0 = s * P
                    nc.tensor.matmul(
                        o,
                        h1_t[3][: KS2[3], c0 : c0 + P],
                        w2_t[3][: KS2[3], 2 * P : H2],
                        start=False,
                        stop=(s == 3),
                        skip_group_check=True,
                    )
                # hf_t[2] (augmented hf44T) is filled by the NEXT chunk's
                # combined transpose
                hf44 = hfpool.tile([F2R + 1, 512], F16, tag="hf44")
                hf_t.append(hf44)

            return {
                "hf_t": hf_t,
                "b0": b0,
                "nb": nb,
                "ns": ns,
                "ptbm": ptbm,
                "w3s": w3s,
            }

        def emit_l3(st, flush=False):
            hf_t, b0, nb, ns = st["hf_t"], st["b0"], st["nb"], st["ns"]
            w3c = st["w3s"]
            pb = nb if ns == 1 else P  # partitions live in the last slice
            if flush:
                # the drain chunk's L3 borrows a ps2 bank (already evicted)
                # instead of waiting for the previous tanh's ps3 read
                pt3 = ps2.tile([P, 512], F32, tag="ps2", name="ps3_flush")
            else:
                pt3 = ps3.tile([P, 4 * A], F32, tag="ps3")
            # ki-outer: the hf44T-dependent (ki=2) passes come last so the
            # transpose/copy chain never stalls the earlier passes
            n_mm = ns * NK3
            i = 0
            for ki in range(NK3):
                ks = KS3[ki]
                for s in range(ns):
                    c0, c1 = s * P, min((s + 1) * P, nb)
                    o = pt3[: c1 - c0, s * A : s * A + A]
                    nc.tensor.matmul(
                        o,
                        hf_t[ki][:ks, c0:c1],
                        w3c[ki][:ks, :A],
                        start=(i == 0),
                        stop=(i == n_mm - 1),
                        skip_group_check=True,
                    )
                    i += 1
            ot = opool.tile([P, 4, A], F32, tag="o")
            nc.scalar.activation(ot[:pb, :ns, :], pt3[:pb, : ns * A], Tanh)
            s0 = b0 // P
            # SP ring: x prefetches leave it nearly idle, and the drain's
            # tanh/eviction chain never waits behind a DMA on ACT
            nc.sync.dma_start(out=out[:pb, s0 : s0 + ns, :], in_=ot[:pb, :ns, :])

        def emit_all():
            prev = None
            for ci, (b0, nb) in enumerate(bchunks):
                prev = emit_chunk(ci, b0, nb, prev)
            emit_l3(prev, flush=True)

        if reps > 1:
            with tc.For_i(0, reps, 1):
                emit_all()
        else:
            emit_all()
    return _legalize_wait_counts(nc) if legalize else nc


def _get_nc(BM):
    key = (BM, _plan.get(BM, 0))
    if key not in _nc_cache:
        _nc_cache[key] = _build(BM)
    return _nc_cache[key]


def pack_k(mat, nk):
    # [K, N] -> zero-pad K to nk*128 -> [128, nk, N] with row j*128+p of the
    # original at [p, j, :] (zero rows contribute nothing to the contraction)
    kk, nn = mat.shape
    pad = np.zeros((nk * P, nn), np.float16)
    pad[:kk] = mat.astype(np.float16)
    return np.ascontiguousarray(pad.reshape(nk, P, nn).transpose(1, 0, 2))


def _aug(mat, row):
    # append a bias row to the contraction dim
    return np.concatenate([mat, row.reshape(1, -1)], axis=0)


def kernel(state, idx, W1, b1, W2, b2, W3, b3):
    global last_run
    state = np.asarray(state, dtype=np.float32)
    idx = np.asarray(idx)
    W1 = np.asarray(W1, dtype=np.float32)
    b1 = np.asarray(b1, dtype=np.float32)
    W2 = np.asarray(W2, dtype=np.float32)
    b2 = np.asarray(b2, dtype=np.float32)
    W3 = np.asarray(W3, dtype=np.float32)
    b3 = np.asarray(b3, dtype=np.float32)
    B = state.shape[0]

    # Host-side routing: idx is sorted in the reference workload; fall back to
    # a stable argsort if not, so grouping stays correct for any input.
    idx_i = idx.astype(np.int64)
    perm = None
    if np.any(np.diff(idx_i) < 0):
        perm = np.argsort(idx_i, kind="stable")
        idx_i = idx_i[perm]
        state = state[perm]
    assert idx_i.min() >= 0 and idx_i.max() < G, "idx out of range [0, G)"
    counts = np.bincount(idx_i, minlength=G)[:G]
    offs = np.concatenate([[0], np.cumsum(counts)])

    BM = max(512, int(-(-counts.max() // P) * P))  # round up to 128 rows

    # Overflow rebalancing plan: hot experts' rows beyond A_CAP go to other
    # cores' B-column donor chunk, shrinking every core's stream from BM to
    # A_CAP + B.
    plan_B = 0
    pieces = []  # (donor_expert, start_row_within_expert, n_rows) per core
    if counts.max() > A_CAP:
        ov = [(g, int(c) - A_CAP) for g, c in enumerate(counts) if c > A_CAP]
        for Bc in (32, 64, 96, 128):
            if sum(-(-o // Bc) for _, o in ov) <= G:
                plan_B = Bc
                break
        if plan_B and A_CAP + plan_B < BM:
            for g, o in ov:
                s = A_CAP
                while s < A_CAP + o:
                    n = min(plan_B, A_CAP + o - s)
                    pieces.append((g, s, n))
                    s += n
        else:
            plan_B = 0
    if plan_B:
        _plan[BM] = plan_B
    else:
        _plan.pop(BM, None)
    nc = _get_nc(BM)
    XW = A_CAP + plan_B if plan_B else BM
    NS = (A_CAP // P + 1) if plan_B else BM // P

    # W2 augmented with the b2 row (the kernel's KS2[-1] = 17 rows cover
    # h1[384:400] + the ones row of h1remT)
    w2p = pack_k(_aug(W2, b2), NK2)
    identity = np.eye(P, dtype=np.float16)

    w1p = [pack_k(_aug(W1[g], b1[g]), NK1) for g in range(G)]
    w3p = [pack_k(_aug(W3[g], b3[g]), NK3) for g in range(G)]
    in_maps = []
    for g in range(G):
        nown = min(int(counts[g]), A_CAP) if plan_B else int(counts[g])
        seg = state[offs[g] : offs[g] + nown]
        xg = np.zeros((D + 1, XW), np.float32)
        xg[:D, : seg.shape[0]] = seg.T
        xg[D, :] = 1.0  # ones row -> b1 via W1's augmented row
        m = {
            "xP": None,
            "w1": w1p[g],
            "w2": w2p,
            "w3": w3p[g],
            "ident": identity,
        }
        if plan_B:
            d = g  # donor defaults to self (empty piece)
            if g < len(pieces):
                d, s0r, n = pieces[g]
                prows = state[offs[d] + s0r : offs[d] + s0r + n]
                xg[:D, A_CAP : A_CAP + n] = prows.T
            m["w1b"] = w1p[d]
            m["w3b"] = w3p[d]
        m["xP"] = pack_k(xg, NK1)
        in_maps.append(m)

    globals()["_last_in_maps"] = in_maps
    try:
        last_run = run_bass_kernel_spmd(nc, in_maps, list(range(NCORES)))
    except ModuleNotFoundError:
        # BASS_TRACE set in an env without the axon NTFF hook: retry untraced
        import os

        os.environ["BASS_NEVER_TRACE"] = "1"
        last_run = run_bass_kernel_spmd(nc, in_maps, list(range(NCORES)))

    out = np.empty((B, A), np.float32)
    for g in range(G):
        og = np.asarray(last_run.results[g]["out"])  # [P, NS, A]
        rows = og.transpose(1, 0, 2).reshape(NS * P, A)
        nown = min(int(counts[g]), A_CAP) if plan_B else int(counts[g])
        out[offs[g] : offs[g] + nown] = rows[:nown]
        if plan_B and g < len(pieces):
            d, s0r, n = pieces[g]
            out[offs[d] + s0r : offs[d] + s0r + n] = rows[A_CAP : A_CAP + n]
    if perm is not None:
        inv = np.empty_like(perm)
        inv[perm] = np.arange(B)
        out = out[inv]
    return out
